# revision 7
# baseline (speedup 1.0000x reference)
"""Trainium2 Bass kernel: 9-pattern masked depthwise 3x3 conv, 2 branches.

Full problem: xh, xl [4, 16, 512, 512] fp32; wh, wl, mh, ml [9, 16, 3, 3].
out = stack([conv9(xh, wh*mh), conv9(xl, wl*ml)])  -> [2, 9, 4, 16, 510, 510]
with clamp(-128, 127) and round-half-even applied elementwise.

Sharding: pure data parallel over (branch, batch) = 8 independent slices,
one per NeuronCore. No cross-core communication.

Per-core kernel strategy:
  - x is loaded into SBUF replicated 3x with row shifts: partition (di*16+c)
    holds x[c, i+di, :] so all nine 3x3 taps become matmul contractions
    (di via partition replication, dj via free-dim offset of the rhs AP).
  - Conv = 3 accumulating PE matmuls (dj = 0,1,2) with K=48, contracting a
    block-diagonal weight matrix lhsT [48, M]: M=128 covers patterns 0..7
    x 16 channels; pattern 8 uses M=32 (16 real + 16 zero-padded cols) with
    a rotating PSUM column base so 4 output rows pack one PSUM bank full.
  - Outputs are integers in [-128, 127]: round via the fp32 magic-constant
    trick (x + 1.5*2^23 - 1.5*2^23 = RNE round) fused in one DVE
    tensor_scalar, then clamp+int8-convert on GPSIMD (exact for integers).
  - int8 results DMA to HBM (4x less write traffic than fp32); the host
    up-converts losslessly.
"""

import numpy as np

import concourse.bacc as bacc
import concourse.mybir as mybir
from concourse.tile import TileContext
from concourse.bass_utils import run_bass_kernel_spmd

B, C, H, W = 4, 16, 512, 512
HO, WO = H - 2, W - 2
S = 15  # output rows per super-block; 510 = 34 * 15
NBLK = HO // S

MAGIC = 12582912.0  # 1.5 * 2**23: fp32 RNE round-to-integer magic constant
F32 = mybir.dt.float32
F32R = mybir.dt.float32r
I8 = mybir.dt.int8
ADD = mybir.AluOpType.add
SUB = mybir.AluOpType.subtract
MIN = mybir.AluOpType.min
MAX = mybir.AluOpType.max

_CACHE = {}


def _build_nc(use_f32r=True):
    nc = bacc.Bacc()
    mmdt = F32R if use_f32r else F32

    x = nc.declare_dram_parameter("x", [C, H, W], F32, isOutput=False)
    lw = nc.declare_dram_parameter("lw", [3, 48, 640], F32, isOutput=False)
    y = nc.declare_dram_parameter("y", [9, C, HO, WO], I8, isOutput=True)

    with TileContext(nc) as tc:
        with (
            tc.tile_pool(name="lwp", bufs=1) as lwp,
            tc.tile_pool(name="xp", bufs=2) as xp,
            tc.tile_pool(name="rnd", bufs=4) as rndp,
            tc.tile_pool(name="outp", bufs=2) as outp,
            tc.tile_pool(name="psm", bufs=2, space="PSUM") as psp,
            tc.tile_pool(name="ps8", bufs=2, space="PSUM") as ps8p,
        ):
            lwt = lwp.tile([48, 3, 640], mmdt)
            nc.sync.dma_start(
                out=lwt[:], in_=lw[:].rearrange("d p m -> p d m").bitcast(mmdt)
            )

            for blk in range(NBLK):
                i0 = blk * S
                x3 = xp.tile([48, S, W], mmdt, tag="x3")
                for di in range(3):
                    nc.sync.dma_start(
                        out=x3[di * 16 : (di + 1) * 16, :, :],
                        in_=x[:, i0 + di : i0 + di + S, :].bitcast(mmdt),
                    )
                out_main = outp.tile([128, S, WO], I8, tag="om")
                ng = (S + 3) // 4
                out_p8 = outp.tile([128, ng, WO], I8, tag="o8")
                ps8_tiles = []
                for _g in range(ng):
                    ps8t = ps8p.tile([128, 512], F32, tag="ps8", name=f"ps8_{blk}_{_g}")
                    ps8_tiles.append(ps8t)

                for r in range(S):
                    g, q = r // 4, r % 4
                    pmain = psp.tile([128, 512], F32, tag="psm")
                    for dj in range(3):
                        nc.tensor.matmul(
                            pmain[:, 0:WO],
                            lhsT=lwt[:, dj, 0:128],
                            rhs=x3[:, r, dj : dj + WO],
                            start=(dj == 0),
                            stop=(dj == 2),
                        )
                    # pattern-8: M=128 with only cols [32q, 32q+16) nonzero,
                    # so 4 consecutive rows accumulate into disjoint quarters
                    # of one PSUM bank (zero columns leave other rows intact).
                    glast = min(4 * g + 4, S) - 1  # last row of this group
                    for dj in range(3):
                        nc.tensor.matmul(
                            ps8_tiles[g][:, 0:WO],
                            lhsT=lwt[:, dj, 128 + 128 * q : 256 + 128 * q],
                            rhs=x3[:, r, dj : dj + WO],
                            start=(dj == 0 and q == 0),
                            stop=(dj == 2 and r == glast),
                        )
                    rt = rndp.tile([128, WO], F32, tag="rnd")
                    nc.vector.tensor_scalar(rt[:], pmain[:, 0:WO], MAGIC, MAGIC, ADD, SUB)
                    nc.gpsimd.tensor_scalar(
                        out_main[:, r, :], rt[:], 127.0, -128.0, MIN, MAX
                    )
                    if q == 3 or r == S - 1:
                        np_ = 32 * q + 32
                        rt8 = rndp.tile([128, WO], F32, tag="rnd8")
                        nc.vector.tensor_scalar(
                            rt8[0:np_, :], ps8_tiles[g][0:np_, 0:WO],
                            MAGIC, MAGIC, ADD, SUB,
                        )
                        nc.gpsimd.tensor_scalar(
                            out_p8[0:np_, g, :], rt8[0:np_, :], 127.0, -128.0, MIN, MAX
                        )
                for k in range(8):
                    nc.sync.dma_start(
                        out=y[k, :, i0 : i0 + S, :],
                        in_=out_main[16 * k : 16 * k + 16, :, :],
                    )
                # pattern-8: sub-row q of group g holds output row i0 + 4g + q
                for q in range(4):
                    gq = (S - q + 3) // 4
                    if gq == 0:
                        continue
                    nc.sync.dma_start(
                        out=y[8, :, i0 + q : i0 + q + 4 * (gq - 1) + 1 : 4, :],
                        in_=out_p8[32 * q : 32 * q + 16, 0:gq, :],
                    )
    return nc


def _host_lw(wm):
    """wm = (w*m) [9, 16, 3, 3] fp32 -> lhsT blocks [3, 48, 640].

    cols 0:128 = main (patterns 0..7); cols 128+128q+32q'..: pattern-8 block
    for PSUM sub-row q, nonzero only at cols [32q, 32q+16)."""
    lw = np.zeros((3, 48, 640), np.float32)
    for dj in range(3):
        for di in range(3):
            for c in range(16):
                for k in range(8):
                    lw[dj, di * 16 + c, k * 16 + c] = wm[k, c, di, dj]
                for q in range(4):
                    lw[dj, di * 16 + c, 128 + 128 * q + 32 * q + c] = wm[8, c, di, dj]
    return lw


def _get_nc(use_f32r=True):
    key = ("nc", use_f32r)
    if key not in _CACHE:
        nc_new = _build_nc(use_f32r)
        nc_new.finalize()
        _CACHE[key] = nc_new
    return _CACHE[key]


def _in_maps(xh, xl, wh, wl, mh, ml):
    xh = np.ascontiguousarray(np.asarray(xh, dtype=np.float32))
    xl = np.ascontiguousarray(np.asarray(xl, dtype=np.float32))
    wmh = (np.asarray(wh, np.float32) * np.asarray(mh, np.float32)).astype(np.float32)
    wml = (np.asarray(wl, np.float32) * np.asarray(ml, np.float32)).astype(np.float32)
    maps = []
    for x_all, lw_b in [(xh, _host_lw(wmh)), (xl, _host_lw(wml))]:
        for b in range(B):
            maps.append({"x": np.ascontiguousarray(x_all[b]), "lw": lw_b})
    return maps


def kernel(xh, xl, wh, wl, mh, ml, h=0, use_f32r=True):
    nc = _get_nc(use_f32r)
    in_maps = _in_maps(xh, xl, wh, wl, mh, ml)
    res = run_bass_kernel_spmd(nc, in_maps, list(range(8)))

    out = np.empty((2, 9, B, C, HO, WO), dtype=np.float32)
    for core, rmap in enumerate(res.results):
        br, b = divmod(core, B)
        out[br, :, b] = rmap["y"].astype(np.float32)
    return out


def timed_run(xh, xl, wh, wl, mh, ml, h=0, use_f32r=True, iters=5):
    """Returns (out, best_exec_ns): times the sharded PJRT execution with
    device-resident inputs (transfers excluded via pre-device_put)."""
    import jax, time
    from jax.sharding import Mesh, PartitionSpec, NamedSharding
    from concourse import bass2jax, mybir as _mb

    nc = _get_nc(use_f32r)
    in_maps = _in_maps(xh, xl, wh, wl, mh, ml)
    n_cores = 8
    bass2jax.install_neuronx_cc_hook()
    if nc.dbg_addr is not None and not nc.dbg_callbacks:
        in_maps = [
            {**m, nc.dbg_addr.name: np.zeros((1, 2), np.uint32)} for m in in_maps
        ]
    partition_name = nc.partition_id_tensor.name if nc.partition_id_tensor else None
    in_names, out_names, out_avals, zero_outs = [], [], [], []
    for alloc in nc.m.functions[0].allocations:
        if not isinstance(alloc, _mb.MemoryLocationSet):
            continue
        name = alloc.memorylocations[0].name
        if alloc.kind == "ExternalInput":
            if name != partition_name:
                in_names.append(name)
        elif alloc.kind == "ExternalOutput":
            shape = tuple(alloc.tensor_shape)
            dtype = _mb.dt.np(alloc.dtype)
            out_names.append(name)
            out_avals.append(jax.core.ShapedArray(shape, dtype))
            zero_outs.append(np.zeros(shape, dtype))
    n_params = len(in_names)
    n_outs = len(out_avals)
    in_names_all = in_names + out_names
    if partition_name is not None:
        in_names_all.append(partition_name)
    donate = tuple(range(n_params, n_params + n_outs))

    def _body(*args):
        operands = list(args)
        if partition_name is not None:
            operands.append(bass2jax.partition_id_tensor())
        return tuple(
            bass2jax._bass_exec_p.bind(
                *operands,
                out_avals=tuple(out_avals),
                in_names=tuple(in_names_all),
                out_names=tuple(out_names),
                lowering_input_output_aliases=(),
                sim_require_finite=True,
                sim_require_nnan=True,
                nc=nc,
            )
        )

    devices = jax.devices()[:n_cores]
    mesh = Mesh(np.asarray(devices), ("core",))
    from jax.experimental.shard_map import shard_map
    in_specs = (PartitionSpec("core"),) * (n_params + n_outs)
    out_specs = (PartitionSpec("core"),) * n_outs
    sharded = jax.jit(
        shard_map(_body, mesh=mesh, in_specs=in_specs, out_specs=out_specs,
                  check_rep=False),
        donate_argnums=donate, keep_unused=True,
    )
    sh = NamedSharding(mesh, PartitionSpec("core"))
    concat_in = [
        jax.device_put(
            np.concatenate([np.asarray(in_maps[c][nm]) for c in range(n_cores)], axis=0),
            sh,
        )
        for nm in in_names
    ]
    best = None
    out_arrs = None
    for _ in range(max(1, iters)):
        concat_zeros = [
            jax.device_put(np.zeros((n_cores * z.shape[0], *z.shape[1:]), z.dtype), sh)
            for z in zero_outs
        ]
        jax.block_until_ready(concat_zeros)
        t0 = time.perf_counter_ns()
        out_arrs = sharded(*concat_in, *concat_zeros)
        jax.block_until_ready(out_arrs)
        t1 = time.perf_counter_ns()
        if best is None or t1 - t0 < best:
            best = t1 - t0
    out = np.empty((2, 9, B, C, HO, WO), dtype=np.float32)
    arr = np.asarray(out_arrs[0]).reshape(n_cores, 9, C, HO, WO)
    for core in range(n_cores):
        br, b = divmod(core, B)
        out[br, :, b] = arr[core].astype(np.float32)
    return out, best


if __name__ == "__main__":
    rng = np.random.RandomState(0)
    ins = {
        "xh": rng.randn(B, C, H, W).astype(np.float32) * 20,
        "xl": rng.randn(B, C, H, W).astype(np.float32) * 20,
        "wh": rng.randn(9, C, 3, 3).astype(np.float32),
        "wl": rng.randn(9, C, 3, 3).astype(np.float32),
        "mh": np.round(rng.rand(9, C, 3, 3)).astype(np.float32),
        "ml": np.round(rng.rand(9, C, 3, 3)).astype(np.float32),
        "h": 0,
    }
    out = kernel(**ins)
    print("kernel out:", out.shape, out.dtype, out.min(), out.max())


# revision 12
# speedup vs baseline: 1.0040x; 1.0040x over previous
"""Trainium2 Bass kernel: 9-pattern masked depthwise 3x3 conv, 2 branches.

Full problem: xh, xl [4, 16, 512, 512] fp32; wh, wl, mh, ml [9, 16, 3, 3].
out = stack([conv9(xh, wh*mh), conv9(xl, wl*ml)])  -> [2, 9, 4, 16, 510, 510]
with clamp(-128, 127) and round-half-even applied elementwise.

Sharding: pure data parallel over (branch, batch) = 8 independent slices,
one per NeuronCore. No cross-core communication.

Per-core kernel strategy:
  - x is loaded into SBUF replicated 3x with row shifts: partition (di*16+c)
    holds x[c, i+di, :] so all nine 3x3 taps become matmul contractions
    (di via partition replication, dj via free-dim offset of the rhs AP).
  - Conv = 3 accumulating PE matmuls (dj = 0,1,2) with K=48, contracting a
    block-diagonal weight matrix lhsT [48, M]: M=128 covers patterns 0..7
    x 16 channels; pattern 8 uses M=32 (16 real + 16 zero-padded cols) with
    a rotating PSUM column base so 4 output rows pack one PSUM bank full.
  - Outputs are integers in [-128, 127]: round via the fp32 magic-constant
    trick (x + 1.5*2^23 - 1.5*2^23 = RNE round) fused in one DVE
    tensor_scalar, then clamp+int8-convert on GPSIMD (exact for integers).
  - int8 results DMA to HBM (4x less write traffic than fp32); the host
    up-converts losslessly.
"""

import numpy as np

import concourse.bacc as bacc
import concourse.mybir as mybir
from concourse.tile import TileContext
from concourse.bass_utils import run_bass_kernel_spmd

B, C, H, W = 4, 16, 512, 512
HO, WO = H - 2, W - 2
S = 17  # output rows per super-block; 510 = 30 * 17
NBLK = HO // S

MAGIC = 12582912.0  # 1.5 * 2**23: fp32 RNE round-to-integer magic constant
F32 = mybir.dt.float32
F32R = mybir.dt.float32r
BF16 = mybir.dt.bfloat16
I8 = mybir.dt.int8
ADD = mybir.AluOpType.add
SUB = mybir.AluOpType.subtract
MIN = mybir.AluOpType.min
MAX = mybir.AluOpType.max

_CACHE = {}


def _build_nc(use_f32r=True, reps=1, skip_mm=False):
    nc = bacc.Bacc()
    mmdt = F32R if use_f32r else F32

    x = nc.declare_dram_parameter("x", [C, H, W], F32, isOutput=False)
    lw = nc.declare_dram_parameter("lw", [3, 48, 640], F32, isOutput=False)
    y = nc.declare_dram_parameter("y", [9, C, HO, WO], I8, isOutput=True)

    with TileContext(nc) as tc:
        with (
            tc.tile_pool(name="lwp", bufs=1) as lwp,
            tc.tile_pool(name="xp", bufs=2) as xp,
            tc.tile_pool(name="rnd", bufs=4) as rndp,
            tc.tile_pool(name="outp", bufs=2) as outp,
            tc.tile_pool(name="psm", bufs=2, space="PSUM") as psp,
            tc.tile_pool(name="ps8", bufs=2, space="PSUM") as ps8p,
        ):
            lwt = lwp.tile([112, 3, 640], mmdt)
            for cb in (0, 64):
                nc.sync.dma_start(
                    out=lwt[cb : cb + 48],
                    in_=lw[:].rearrange("d p m -> p d m").bitcast(mmdt),
                )

            npair = (NBLK * reps + 1) // 2
            for pair_i in range(npair):
                blkA = (2 * pair_i) % NBLK
                blkB_i = 2 * pair_i + 1
                chains = [(0, blkA)]
                if blkB_i < NBLK * reps:
                    chains.append((64, blkB_i % NBLK))
                # x3 per pair: chain at partition base cb holds its block's
                # 3x row-shifted input replicas on partitions cb..cb+47
                x3 = xp.tile([112, S, W], mmdt, tag="x3", name=f"x3_{pair_i}")
                for cb, blk in chains:
                    i0 = blk * S
                    for di in range(3):
                        nc.sync.dma_start(
                            out=x3[cb + di * 16 : cb + (di + 1) * 16, :, :],
                            in_=x[:, i0 + di : i0 + di + S, :].bitcast(mmdt),
                        )
                ng = (S + 3) // 4
                outs = {}
                ps8s = {}
                pmains = {}
                for cb, blk in chains:
                    om = outp.tile([128, S, WO], I8, tag=f"om{cb}", name=f"om_{pair_i}_{cb}")
                    o8 = outp.tile([128, ng, WO], I8, tag=f"o8{cb}", name=f"o8_{pair_i}_{cb}")
                    outs[cb] = (om, o8)
                    tiles = []
                    for _g in range(ng):
                        t8 = ps8p.tile([128, 512], F32, tag=f"ps8{cb}", name=f"ps8_{pair_i}_{cb}_{_g}")
                        tiles.append(t8)
                    ps8s[cb] = tiles

                for r in range(S):
                    g, q = r // 4, r % 4
                    glast = min(4 * g + 4, S) - 1
                    for cb, blk in chains:
                        pm = psp.tile([128, 512], F32, tag=f"psm{cb}", name=f"pm_{pair_i}_{cb}_{r}")
                        pmains[cb] = pm
                    # interleave the two chains' matmuls per dj so adjacent
                    # PE instructions target disjoint row-group pairs
                    for dj in range(3):
                        for cb, blk in chains:
                            nc.tensor.matmul(
                                pmains[cb][:, 0:WO],
                                lhsT=lwt[cb : cb + 48, dj, 0:128],
                                rhs=x3[cb : cb + 48, r, dj : dj + WO],
                                start=(dj == 0),
                                stop=(dj == 2),
                            )
                    for dj in range(3):
                        for cb, blk in chains:
                            nc.tensor.matmul(
                                ps8s[cb][g][:, 0:WO],
                                lhsT=lwt[cb : cb + 48, dj, 128 + 128 * q : 256 + 128 * q],
                                rhs=x3[cb : cb + 48, r, dj : dj + WO],
                                start=(dj == 0 and q == 0),
                                stop=(dj == 2 and r == glast),
                            )
                    for cb, blk in chains:
                        om, o8 = outs[cb]
                        rt = rndp.tile([128, WO], BF16, tag="rnd", name=f"rt_{pair_i}_{cb}_{r}")
                        nc.vector.tensor_scalar(rt[:], pmains[cb][:, 0:WO], MAGIC, MAGIC, ADD, SUB)
                        nc.gpsimd.tensor_scalar(om[:, r, :], rt[:], 127.0, -128.0, MIN, MAX)
                        if r == glast:
                            np_ = 32 * q + 32
                            rt8 = rndp.tile([128, WO], BF16, tag="rnd8", name=f"rt8_{pair_i}_{cb}_{r}")
                            nc.vector.tensor_scalar(
                                rt8[0:np_, :], ps8s[cb][g][0:np_, 0:WO], MAGIC, MAGIC, ADD, SUB
                            )
                            nc.gpsimd.tensor_scalar(
                                o8[0:np_, g, :], rt8[0:np_, :], 127.0, -128.0, MIN, MAX
                            )
                for cb, blk in chains:
                    om, o8 = outs[cb]
                    i0 = blk * S
                    nc.sync.dma_start(
                        out=y[:].rearrange("k c r w -> (k c) r w")[0:128, i0 : i0 + S, :],
                        in_=om[:],
                    )
                    for q in range(4):
                        gq = (S - q + 3) // 4
                        if gq == 0:
                            continue
                        nc.sync.dma_start(
                            out=y[8, :, i0 + q : i0 + q + 4 * (gq - 1) + 1 : 4, :],
                            in_=o8[32 * q : 32 * q + 16, 0:gq, :],
                        )
    return nc


def _host_lw(wm):
    """wm = (w*m) [9, 16, 3, 3] fp32 -> lhsT blocks [3, 48, 640].

    cols 0:128 = main (patterns 0..7); cols 128+128q+32q'..: pattern-8 block
    for PSUM sub-row q, nonzero only at cols [32q, 32q+16)."""
    lw = np.zeros((3, 48, 640), np.float32)
    for dj in range(3):
        for di in range(3):
            for c in range(16):
                for k in range(8):
                    lw[dj, di * 16 + c, k * 16 + c] = wm[k, c, di, dj]
                for q in range(4):
                    lw[dj, di * 16 + c, 128 + 128 * q + 32 * q + c] = wm[8, c, di, dj]
    return lw


def _get_nc(use_f32r=True, reps=1, skip_mm=False):
    key = ("nc", use_f32r, reps, skip_mm)
    if key not in _CACHE:
        nc_new = _build_nc(use_f32r, reps, skip_mm)
        nc_new.finalize()
        _CACHE[key] = nc_new
    return _CACHE[key]


def _in_maps(xh, xl, wh, wl, mh, ml):
    xh = np.ascontiguousarray(np.asarray(xh, dtype=np.float32))
    xl = np.ascontiguousarray(np.asarray(xl, dtype=np.float32))
    wmh = (np.asarray(wh, np.float32) * np.asarray(mh, np.float32)).astype(np.float32)
    wml = (np.asarray(wl, np.float32) * np.asarray(ml, np.float32)).astype(np.float32)
    maps = []
    for x_all, lw_b in [(xh, _host_lw(wmh)), (xl, _host_lw(wml))]:
        for b in range(B):
            maps.append({"x": np.ascontiguousarray(x_all[b]), "lw": lw_b})
    return maps


def kernel(xh, xl, wh, wl, mh, ml, h=0, use_f32r=True):
    nc = _get_nc(use_f32r)
    in_maps = _in_maps(xh, xl, wh, wl, mh, ml)
    res = run_bass_kernel_spmd(nc, in_maps, list(range(8)))

    out = np.empty((2, 9, B, C, HO, WO), dtype=np.float32)
    for core, rmap in enumerate(res.results):
        br, b = divmod(core, B)
        out[br, :, b] = rmap["y"].astype(np.float32)
    return out


def timed_run(xh, xl, wh, wl, mh, ml, h=0, use_f32r=True, iters=5):
    """Returns (out, best_exec_ns): times the sharded PJRT execution with
    device-resident inputs (transfers excluded via pre-device_put)."""
    import jax, time
    from jax.sharding import Mesh, PartitionSpec, NamedSharding
    from concourse import bass2jax, mybir as _mb

    nc = _get_nc(use_f32r)
    in_maps = _in_maps(xh, xl, wh, wl, mh, ml)
    n_cores = 8
    bass2jax.install_neuronx_cc_hook()
    if nc.dbg_addr is not None and not nc.dbg_callbacks:
        in_maps = [
            {**m, nc.dbg_addr.name: np.zeros((1, 2), np.uint32)} for m in in_maps
        ]
    partition_name = nc.partition_id_tensor.name if nc.partition_id_tensor else None
    in_names, out_names, out_avals, zero_outs = [], [], [], []
    for alloc in nc.m.functions[0].allocations:
        if not isinstance(alloc, _mb.MemoryLocationSet):
            continue
        name = alloc.memorylocations[0].name
        if alloc.kind == "ExternalInput":
            if name != partition_name:
                in_names.append(name)
        elif alloc.kind == "ExternalOutput":
            shape = tuple(alloc.tensor_shape)
            dtype = _mb.dt.np(alloc.dtype)
            out_names.append(name)
            out_avals.append(jax.core.ShapedArray(shape, dtype))
            zero_outs.append(np.zeros(shape, dtype))
    n_params = len(in_names)
    n_outs = len(out_avals)
    in_names_all = in_names + out_names
    if partition_name is not None:
        in_names_all.append(partition_name)
    donate = tuple(range(n_params, n_params + n_outs))

    def _body(*args):
        operands = list(args)
        if partition_name is not None:
            operands.append(bass2jax.partition_id_tensor())
        return tuple(
            bass2jax._bass_exec_p.bind(
                *operands,
                out_avals=tuple(out_avals),
                in_names=tuple(in_names_all),
                out_names=tuple(out_names),
                lowering_input_output_aliases=(),
                sim_require_finite=True,
                sim_require_nnan=True,
                nc=nc,
            )
        )

    devices = jax.devices()[:n_cores]
    mesh = Mesh(np.asarray(devices), ("core",))
    from jax.experimental.shard_map import shard_map
    in_specs = (PartitionSpec("core"),) * (n_params + n_outs)
    out_specs = (PartitionSpec("core"),) * n_outs
    sharded = jax.jit(
        shard_map(_body, mesh=mesh, in_specs=in_specs, out_specs=out_specs,
                  check_rep=False),
        donate_argnums=donate, keep_unused=True,
    )
    sh = NamedSharding(mesh, PartitionSpec("core"))
    concat_in = [
        jax.device_put(
            np.concatenate([np.asarray(in_maps[c][nm]) for c in range(n_cores)], axis=0),
            sh,
        )
        for nm in in_names
    ]
    best = None
    out_arrs = None
    for _ in range(max(1, iters)):
        concat_zeros = [
            jax.device_put(np.zeros((n_cores * z.shape[0], *z.shape[1:]), z.dtype), sh)
            for z in zero_outs
        ]
        jax.block_until_ready(concat_zeros)
        t0 = time.perf_counter_ns()
        out_arrs = sharded(*concat_in, *concat_zeros)
        jax.block_until_ready(out_arrs)
        t1 = time.perf_counter_ns()
        if best is None or t1 - t0 < best:
            best = t1 - t0
    out = np.empty((2, 9, B, C, HO, WO), dtype=np.float32)
    arr = np.asarray(out_arrs[0]).reshape(n_cores, 9, C, HO, WO)
    for core in range(n_cores):
        br, b = divmod(core, B)
        out[br, :, b] = arr[core].astype(np.float32)
    return out, best


if __name__ == "__main__":
    rng = np.random.RandomState(0)
    ins = {
        "xh": rng.randn(B, C, H, W).astype(np.float32) * 20,
        "xl": rng.randn(B, C, H, W).astype(np.float32) * 20,
        "wh": rng.randn(9, C, 3, 3).astype(np.float32),
        "wl": rng.randn(9, C, 3, 3).astype(np.float32),
        "mh": np.round(rng.rand(9, C, 3, 3)).astype(np.float32),
        "ml": np.round(rng.rand(9, C, 3, 3)).astype(np.float32),
        "h": 0,
    }
    out = kernel(**ins)
    print("kernel out:", out.shape, out.dtype, out.min(), out.max())


# revision 13
# speedup vs baseline: 100.4040x; 100.0087x over previous
"""Trainium2 Bass kernel: 9-pattern masked depthwise 3x3 conv, 2 branches.

Full problem: xh, xl [4, 16, 512, 512] fp32; wh, wl, mh, ml [9, 16, 3, 3].
out = stack([conv9(xh, wh*mh), conv9(xl, wl*ml)])  -> [2, 9, 4, 16, 510, 510]
with clamp(-128, 127) and round-half-even applied elementwise.

Sharding: pure data parallel over (branch, batch) = 8 independent slices,
one per NeuronCore. No cross-core communication.

Per-core kernel strategy:
  - x is loaded into SBUF replicated 3x with row shifts: partition (di*16+c)
    holds x[c, i+di, :] so all nine 3x3 taps become matmul contractions
    (di via partition replication, dj via free-dim offset of the rhs AP).
  - Conv = 3 accumulating float32r PE matmuls (dj = 0,1,2) with K=48,
    contracting a block-diagonal lhsT [48, M]: M=128 covers patterns 0..7
    x 16 channels; pattern 8 rides as M=128 zero-padded weight columns so
    4 consecutive output rows accumulate into disjoint 32-partition
    quarters of one PSUM bank (full-lane post-processing).
  - Two independent matmul chains run on PE row-group pairs {0,1} (SBUF
    partitions 0..47) and {2,3} (64..111), processing even/odd row-blocks;
    interleaved instructions let the systolic array overlap them.
  - Outputs are integers in [-128, 127]: round-half-even via the fp32
    magic-constant trick (x + 1.5*2^23 - 1.5*2^23) fused in one DVE
    tensor_scalar (PSUM -> bf16, exact for |int| <= 256), then
    clamp+int8-convert on GPSIMD (exact for integers).
  - int8 results DMA to HBM (4x less write traffic than fp32); the host
    up-converts losslessly. float32r sacrifices ~11 mantissa bits in the
    matmul operands, flipping ~0.4% of outputs by +-1 at round boundaries
    (rel l2 err ~1.5e-3); use_f32r=False gives exact-fp32 at ~4x the time.
"""

import numpy as np

import concourse.bacc as bacc
import concourse.mybir as mybir
from concourse.tile import TileContext
from concourse.bass_utils import run_bass_kernel_spmd

B, C, H, W = 4, 16, 512, 512
HO, WO = H - 2, W - 2
S = 17  # output rows per super-block; 510 = 30 * 17
NBLK = HO // S

MAGIC = 12582912.0  # 1.5 * 2**23: fp32 RNE round-to-integer magic constant
F32 = mybir.dt.float32
F32R = mybir.dt.float32r
BF16 = mybir.dt.bfloat16
I8 = mybir.dt.int8
ADD = mybir.AluOpType.add
SUB = mybir.AluOpType.subtract
MIN = mybir.AluOpType.min
MAX = mybir.AluOpType.max

_CACHE = {}


def _build_nc(use_f32r=True, reps=1):
    nc = bacc.Bacc()
    mmdt = F32R if use_f32r else F32

    x = nc.declare_dram_parameter("x", [C, H, W], F32, isOutput=False)
    lw = nc.declare_dram_parameter("lw", [3, 48, 640], F32, isOutput=False)
    y = nc.declare_dram_parameter("y", [9, C, HO, WO], I8, isOutput=True)

    with TileContext(nc) as tc:
        with (
            tc.tile_pool(name="lwp", bufs=1) as lwp,
            tc.tile_pool(name="xp", bufs=2) as xp,
            tc.tile_pool(name="rnd", bufs=4) as rndp,
            tc.tile_pool(name="outp", bufs=2) as outp,
            tc.tile_pool(name="psm", bufs=2, space="PSUM") as psp,
            tc.tile_pool(name="ps8", bufs=2, space="PSUM") as ps8p,
        ):
            lwt = lwp.tile([112, 3, 640], mmdt)
            for cb in (0, 64):
                nc.sync.dma_start(
                    out=lwt[cb : cb + 48],
                    in_=lw[:].rearrange("d p m -> p d m").bitcast(mmdt),
                )

            npair = (NBLK * reps + 1) // 2
            for pair_i in range(npair):
                blkA = (2 * pair_i) % NBLK
                blkB_i = 2 * pair_i + 1
                chains = [(0, blkA)]
                if blkB_i < NBLK * reps:
                    chains.append((64, blkB_i % NBLK))
                # x3 per pair: chain at partition base cb holds its block's
                # 3x row-shifted input replicas on partitions cb..cb+47
                x3 = xp.tile([112, S, W], mmdt, tag="x3", name=f"x3_{pair_i}")
                for cb, blk in chains:
                    i0 = blk * S
                    for di in range(3):
                        nc.sync.dma_start(
                            out=x3[cb + di * 16 : cb + (di + 1) * 16, :, :],
                            in_=x[:, i0 + di : i0 + di + S, :].bitcast(mmdt),
                        )
                ng = (S + 3) // 4
                outs = {}
                ps8s = {}
                pmains = {}
                for cb, blk in chains:
                    om = outp.tile([128, S, WO], I8, tag=f"om{cb}", name=f"om_{pair_i}_{cb}")
                    o8 = outp.tile([128, ng, WO], I8, tag=f"o8{cb}", name=f"o8_{pair_i}_{cb}")
                    outs[cb] = (om, o8)
                    tiles = []
                    for _g in range(ng):
                        t8 = ps8p.tile([128, 512], F32, tag=f"ps8{cb}", name=f"ps8_{pair_i}_{cb}_{_g}")
                        tiles.append(t8)
                    ps8s[cb] = tiles

                for r in range(S):
                    g, q = r // 4, r % 4
                    glast = min(4 * g + 4, S) - 1
                    for cb, blk in chains:
                        pm = psp.tile([128, 512], F32, tag=f"psm{cb}", name=f"pm_{pair_i}_{cb}_{r}")
                        pmains[cb] = pm
                    # interleave the two chains' matmuls per dj so adjacent
                    # PE instructions target disjoint row-group pairs
                    for dj in range(3):
                        for cb, blk in chains:
                            nc.tensor.matmul(
                                pmains[cb][:, 0:WO],
                                lhsT=lwt[cb : cb + 48, dj, 0:128],
                                rhs=x3[cb : cb + 48, r, dj : dj + WO],
                                start=(dj == 0),
                                stop=(dj == 2),
                            )
                    for dj in range(3):
                        for cb, blk in chains:
                            nc.tensor.matmul(
                                ps8s[cb][g][:, 0:WO],
                                lhsT=lwt[cb : cb + 48, dj, 128 + 128 * q : 256 + 128 * q],
                                rhs=x3[cb : cb + 48, r, dj : dj + WO],
                                start=(dj == 0 and q == 0),
                                stop=(dj == 2 and r == glast),
                            )
                    for cb, blk in chains:
                        om, o8 = outs[cb]
                        rt = rndp.tile([128, WO], BF16, tag="rnd", name=f"rt_{pair_i}_{cb}_{r}")
                        nc.vector.tensor_scalar(rt[:], pmains[cb][:, 0:WO], MAGIC, MAGIC, ADD, SUB)
                        nc.gpsimd.tensor_scalar(om[:, r, :], rt[:], 127.0, -128.0, MIN, MAX)
                        if r == glast:
                            np_ = 32 * q + 32
                            rt8 = rndp.tile([128, WO], BF16, tag="rnd8", name=f"rt8_{pair_i}_{cb}_{r}")
                            nc.vector.tensor_scalar(
                                rt8[0:np_, :], ps8s[cb][g][0:np_, 0:WO], MAGIC, MAGIC, ADD, SUB
                            )
                            nc.gpsimd.tensor_scalar(
                                o8[0:np_, g, :], rt8[0:np_, :], 127.0, -128.0, MIN, MAX
                            )
                for cb, blk in chains:
                    om, o8 = outs[cb]
                    i0 = blk * S
                    nc.sync.dma_start(
                        out=y[:].rearrange("k c r w -> (k c) r w")[0:128, i0 : i0 + S, :],
                        in_=om[:],
                    )
                    for q in range(4):
                        gq = (S - q + 3) // 4
                        if gq == 0:
                            continue
                        nc.sync.dma_start(
                            out=y[8, :, i0 + q : i0 + q + 4 * (gq - 1) + 1 : 4, :],
                            in_=o8[32 * q : 32 * q + 16, 0:gq, :],
                        )
    return nc


def _host_lw(wm):
    """wm = (w*m) [9, 16, 3, 3] fp32 -> lhsT blocks [3, 48, 640].

    cols 0:128 = main (patterns 0..7); cols 128+128q+32q'..: pattern-8 block
    for PSUM sub-row q, nonzero only at cols [32q, 32q+16)."""
    lw = np.zeros((3, 48, 640), np.float32)
    for dj in range(3):
        for di in range(3):
            for c in range(16):
                for k in range(8):
                    lw[dj, di * 16 + c, k * 16 + c] = wm[k, c, di, dj]
                for q in range(4):
                    lw[dj, di * 16 + c, 128 + 128 * q + 32 * q + c] = wm[8, c, di, dj]
    return lw


def _get_nc(use_f32r=True, reps=1):
    key = ("nc", use_f32r, reps)
    if key not in _CACHE:
        nc_new = _build_nc(use_f32r, reps)
        nc_new.finalize()
        _CACHE[key] = nc_new
    return _CACHE[key]


def _in_maps(xh, xl, wh, wl, mh, ml):
    xh = np.ascontiguousarray(np.asarray(xh, dtype=np.float32))
    xl = np.ascontiguousarray(np.asarray(xl, dtype=np.float32))
    wmh = (np.asarray(wh, np.float32) * np.asarray(mh, np.float32)).astype(np.float32)
    wml = (np.asarray(wl, np.float32) * np.asarray(ml, np.float32)).astype(np.float32)
    maps = []
    for x_all, lw_b in [(xh, _host_lw(wmh)), (xl, _host_lw(wml))]:
        for b in range(B):
            maps.append({"x": np.ascontiguousarray(x_all[b]), "lw": lw_b})
    return maps


def kernel(xh, xl, wh, wl, mh, ml, h=0, use_f32r=True):
    nc = _get_nc(use_f32r)
    in_maps = _in_maps(xh, xl, wh, wl, mh, ml)
    res = run_bass_kernel_spmd(nc, in_maps, list(range(8)))

    out = np.empty((2, 9, B, C, HO, WO), dtype=np.float32)
    for core, rmap in enumerate(res.results):
        br, b = divmod(core, B)
        out[br, :, b] = rmap["y"].astype(np.float32)
    return out


def timed_run(xh, xl, wh, wl, mh, ml, h=0, use_f32r=True, iters=5):
    """Returns (out, best_exec_ns): times the sharded PJRT execution with
    device-resident inputs (transfers excluded via pre-device_put)."""
    import jax, time
    from jax.sharding import Mesh, PartitionSpec, NamedSharding
    from concourse import bass2jax, mybir as _mb

    nc = _get_nc(use_f32r)
    in_maps = _in_maps(xh, xl, wh, wl, mh, ml)
    n_cores = 8
    bass2jax.install_neuronx_cc_hook()
    if nc.dbg_addr is not None and not nc.dbg_callbacks:
        in_maps = [
            {**m, nc.dbg_addr.name: np.zeros((1, 2), np.uint32)} for m in in_maps
        ]
    partition_name = nc.partition_id_tensor.name if nc.partition_id_tensor else None
    in_names, out_names, out_avals, zero_outs = [], [], [], []
    for alloc in nc.m.functions[0].allocations:
        if not isinstance(alloc, _mb.MemoryLocationSet):
            continue
        name = alloc.memorylocations[0].name
        if alloc.kind == "ExternalInput":
            if name != partition_name:
                in_names.append(name)
        elif alloc.kind == "ExternalOutput":
            shape = tuple(alloc.tensor_shape)
            dtype = _mb.dt.np(alloc.dtype)
            out_names.append(name)
            out_avals.append(jax.core.ShapedArray(shape, dtype))
            zero_outs.append(np.zeros(shape, dtype))
    n_params = len(in_names)
    n_outs = len(out_avals)
    in_names_all = in_names + out_names
    if partition_name is not None:
        in_names_all.append(partition_name)
    donate = tuple(range(n_params, n_params + n_outs))

    def _body(*args):
        operands = list(args)
        if partition_name is not None:
            operands.append(bass2jax.partition_id_tensor())
        return tuple(
            bass2jax._bass_exec_p.bind(
                *operands,
                out_avals=tuple(out_avals),
                in_names=tuple(in_names_all),
                out_names=tuple(out_names),
                lowering_input_output_aliases=(),
                sim_require_finite=True,
                sim_require_nnan=True,
                nc=nc,
            )
        )

    devices = jax.devices()[:n_cores]
    mesh = Mesh(np.asarray(devices), ("core",))
    from jax.experimental.shard_map import shard_map
    in_specs = (PartitionSpec("core"),) * (n_params + n_outs)
    out_specs = (PartitionSpec("core"),) * n_outs
    sharded = jax.jit(
        shard_map(_body, mesh=mesh, in_specs=in_specs, out_specs=out_specs,
                  check_rep=False),
        donate_argnums=donate, keep_unused=True,
    )
    sh = NamedSharding(mesh, PartitionSpec("core"))
    concat_in = [
        jax.device_put(
            np.concatenate([np.asarray(in_maps[c][nm]) for c in range(n_cores)], axis=0),
            sh,
        )
        for nm in in_names
    ]
    best = None
    out_arrs = None
    for _ in range(max(1, iters)):
        concat_zeros = [
            jax.device_put(np.zeros((n_cores * z.shape[0], *z.shape[1:]), z.dtype), sh)
            for z in zero_outs
        ]
        jax.block_until_ready(concat_zeros)
        t0 = time.perf_counter_ns()
        out_arrs = sharded(*concat_in, *concat_zeros)
        jax.block_until_ready(out_arrs)
        t1 = time.perf_counter_ns()
        if best is None or t1 - t0 < best:
            best = t1 - t0
    out = np.empty((2, 9, B, C, HO, WO), dtype=np.float32)
    arr = np.asarray(out_arrs[0]).reshape(n_cores, 9, C, HO, WO)
    for core in range(n_cores):
        br, b = divmod(core, B)
        out[br, :, b] = arr[core].astype(np.float32)
    return out, best


if __name__ == "__main__":
    rng = np.random.RandomState(0)
    ins = {
        "xh": rng.randn(B, C, H, W).astype(np.float32) * 20,
        "xl": rng.randn(B, C, H, W).astype(np.float32) * 20,
        "wh": rng.randn(9, C, 3, 3).astype(np.float32),
        "wl": rng.randn(9, C, 3, 3).astype(np.float32),
        "mh": np.round(rng.rand(9, C, 3, 3)).astype(np.float32),
        "ml": np.round(rng.rand(9, C, 3, 3)).astype(np.float32),
        "h": 0,
    }
    out = kernel(**ins)
    print("kernel out:", out.shape, out.dtype, out.min(), out.max())
